# revision 1
# baseline (speedup 1.0000x reference)
"""Expert-parallel MoE ConditionalFeedForward (SwiGLU) for 8 Trainium2 cores.

Math (per token t, selected expert e):
    out[t] = (silu(x[t] @ w1[e].T) * (x[t] @ w3[e].T)) @ w2[e]

Strategy: one expert per NeuronCore (8 experts / 8 cores). The host routes
tokens to experts (gather), each core runs the dense SwiGLU FFN for its
expert's tokens, and the host scatters results back into [T, top_k, D].

On-chip layout keeps the hidden dim H on SBUF partitions throughout
([h, tok] activations), so stage-1 outputs feed stage-2 matmuls with no
transposes. All matmul operands are float32r (full-rate fp32-reduced).
"""

import numpy as np

import concourse.bacc as bacc
import concourse.mybir as mybir
from concourse.bass_utils import run_bass_kernel_spmd
from concourse.tile import TileContext

# Problem constants (nn_ConditionalFeedForward: dim=1024, hidden=2816, 8 experts, top-2)
T = 2048
D = 1024
H = 2816
E = 8
TOPK = 2
ND = D // 128   # 8 d-chunks
NH = H // 128   # 22 h-tiles

F32 = mybir.dt.float32
F32R = mybir.dt.float32r

_BUILD_CACHE: dict[tuple, object] = {}


def _build(npad: int, loop_n: int = 0):
    """Bass program for one core: dense SwiGLU FFN over npad tokens.

    loop_n > 0 wraps the body in a hardware loop (benchmarking only).
    """
    key = (npad, loop_n)
    if key in _BUILD_CACHE:
        return _BUILD_CACHE[key]
    # token chunks: as few as possible, each <=512 (one PSUM bank) and
    # >=256 so float32r matmuls run at full rate
    nchunks = -(-npad // 512)
    base = npad // nchunks
    sizes = [base + (1 if i < npad % nchunks else 0) for i in range(nchunks)]
    chunks, off = [], 0
    for sz in sizes:
        chunks.append((off, sz))
        off += sz

    nc = bacc.Bacc("TRN2", target_bir_lowering=False)
    xt = nc.dram_tensor("xt", [128, ND * npad], F32R, kind="ExternalInput")
    w13 = nc.dram_tensor("w13", [NH, 128, 2 * ND * 128], F32R, kind="ExternalInput")
    w2t = nc.dram_tensor("w2t", [ND, 128, NH * 128], F32R, kind="ExternalInput")
    outt = nc.dram_tensor("outt", [ND, 128, npad], F32, kind="ExternalOutput")

    import contextlib

    with TileContext(nc) as tc:
        with (
            tc.For_i(0, loop_n, 1) if loop_n else contextlib.nullcontext(),
            tc.tile_pool(name="xg", bufs=1) as xg_pool,
            tc.tile_pool(name="w13p", bufs=6) as w13_pool,
            tc.tile_pool(name="w2p", bufs=3) as w2_pool,
            tc.tile_pool(name="tmp", bufs=4) as tmp_pool,
        ):
            x_sb = xg_pool.tile([128, ND * npad], F32R)
            w13_first = w13_pool.tile([128, 2 * ND * 128], F32R, tag="wt", name="wt0")
            HALF = ND * 128
            # interleave so the first matmul group is gated on ~0.8 MB, not 3.2 MB
            nc.sync.dma_start(x_sb[:, 0:npad], xt[:, 0:npad])
            nc.sync.dma_start(w13_first[:, :HALF], w13[0, :, :HALF])
            for d in range(1, ND):
                nc.sync.dma_start(
                    x_sb[:, d * npad:(d + 1) * npad],
                    xt[:, d * npad:(d + 1) * npad],
                )
            nc.sync.dma_start(w13_first[:, HALF:], w13[0, :, HALF:])
            g_sb = xg_pool.tile([128, NH * npad], F32R, tag="g")

            # stage-2 weight prefetch (filled during stage 1)
            w2_tiles = {}

            def load_w2(dt):
                t = w2_pool.tile([128, NH * 128], F32R, name=f"w2_{dt}", tag="w2")
                nc.sync.dma_start(t[:], w2t[dt])
                w2_tiles[dt] = t

            # ---- stage 1: gT[h, n] = silu(w1.T x)[h, n] * (w3.T x)[h, n] ----
            with tc.tile_pool(name="ps1", bufs=3, space="PSUM") as ps1_pool, \
                 tc.tile_pool(name="ps2", bufs=2, space="PSUM") as ps2_pool:
                for h in range(NH):
                    if h in (10, 16):
                        load_w2({10: 0, 16: 1}[h])
                    if h == 0:
                        wt = w13_first
                    else:
                        wt = w13_pool.tile([128, 2 * ND * 128], F32R, tag="wt")
                        nc.sync.dma_start(wt[:, :HALF], w13[h, :, :HALF])
                        nc.sync.dma_start(wt[:, HALF:], w13[h, :, HALF:])
                    ps = {
                        (ci, s): ps1_pool.tile([128, cl], F32, tag=f"ps{ci}{s}",
                                               name=f"ps_{ci}_{s}")
                        for ci, (cs, cl) in enumerate(chunks) for s in range(2)
                    }
                    # s outermost: h=0's s=1 matmuls start after its DMA half
                    order = [(d, ci, s) for s in range(2)
                             for ci in range(len(chunks)) for d in range(ND)]
                    for d, ci, s in order:
                        cs, cl = chunks[ci]
                        nc.tensor.matmul(
                            ps[ci, s][:],
                            wt[:, (s * ND + d) * 128:(s * ND + d + 1) * 128],
                            x_sb[:, d * npad + cs: d * npad + cs + cl],
                            start=(d == 0),
                            stop=(d == ND - 1),
                        )
                    for ci, (cs, cl) in enumerate(chunks):
                        t_silu = tmp_pool.tile([128, cl], F32, tag=f"silu{ci}")
                        nc.scalar.activation(
                            t_silu[:], ps[ci, 0][:], mybir.ActivationFunctionType.Silu
                        )
                        nc.vector.tensor_mul(
                            g_sb[:, h * npad + cs: h * npad + cs + cl],
                            t_silu[:], ps[ci, 1][:],
                        )

                # ---- stage 2: out[dt, n] = sum_h w2[h, dt].T gT[h, n] ----
                for dt in range(ND):
                    if dt + 2 < ND:
                        load_w2(dt + 2)
                    w2_sb = w2_tiles.pop(dt)
                    for ci, (cs, cl) in enumerate(chunks):
                        ps = ps2_pool.tile([128, cl], F32, tag=f"o{ci}", name="o_ps")
                        for hc in range(NH):
                            nc.tensor.matmul(
                                ps[:],
                                w2_sb[:, hc * 128:(hc + 1) * 128],
                                g_sb[:, hc * npad + cs: hc * npad + cs + cl],
                                start=(hc == 0),
                                stop=(hc == NH - 1),
                            )
                        # split the drain: copy+DMA halves overlap the next MMs
                        half = cl // 2
                        for oi, (ho, hl) in enumerate([(0, half), (half, cl - half)]):
                            o_sb = tmp_pool.tile([128, hl], F32, tag=f"ot{ci}{oi}",
                                                 name="o_sb")
                            nc.scalar.copy(o_sb[:], ps[:, ho:ho + hl])
                            nc.sync.dma_start(
                                outt[dt, :, cs + ho:cs + ho + hl], o_sb[:])
    nc.compile()
    _BUILD_CACHE[key] = nc
    return nc


def _route(expert_indices: np.ndarray):
    """Per-expert token lists, padded count, and an inverse position map."""
    toks = []
    for e in range(E):
        mask = (expert_indices == e).any(axis=1)
        toks.append(np.flatnonzero(mask))
    maxc = max(len(tk) for tk in toks)
    npad = max(512, -(-maxc // 8) * 8)
    inv = np.zeros((E, T), dtype=np.int64)
    for e, tk in enumerate(toks):
        inv[e, tk] = np.arange(len(tk))
    return toks, npad, inv


def _run(inputs, trace=False):
    x = np.ascontiguousarray(inputs["x"], dtype=np.float32)
    idx = np.asarray(inputs["expert_indices"])
    w1 = np.asarray(inputs["w1"], dtype=np.float32)
    w2 = np.asarray(inputs["w2"], dtype=np.float32)
    w3 = np.asarray(inputs["w3"], dtype=np.float32)

    toks, npad, inv = _route(idx)
    nc = _build(npad)

    in_maps = []
    for e in range(E):
        tk = toks[e]
        xg = np.zeros((npad, D), dtype=np.float32)
        xg[: len(tk)] = x[tk]
        # xt[i, d*npad + n] = xg[n, d*128 + i]
        xt = np.ascontiguousarray(
            xg.reshape(npad, ND, 128).transpose(2, 1, 0).reshape(128, ND * npad)
        )
        # w13[h, i, (s*ND + d)*128 + j] = w_s[h*128 + j, d*128 + i]
        w13 = np.stack([w1[e], w3[e]]).reshape(2, NH, 128, ND, 128)
        w13 = np.ascontiguousarray(
            w13.transpose(1, 4, 0, 3, 2).reshape(NH, 128, 2 * ND * 128)
        )
        # w2t[dt, i, hc*128 + j] = w2[hc*128 + i, dt*128 + j]
        w2e = np.ascontiguousarray(
            w2[e].reshape(NH, 128, ND, 128).transpose(2, 1, 0, 3)
            .reshape(ND, 128, NH * 128)
        )
        in_maps.append({"xt": xt, "w13": w13, "w2t": w2e})

    res = run_bass_kernel_spmd(
        nc, in_maps, core_ids=list(range(E)), trace=trace,
        **({"stitch_traces": True} if trace else {}),
    )

    # outs[e, n, dd] = outt[dt, i, n] with dd = dt*128 + i
    outs = np.empty((E, npad, D), dtype=np.float32)
    for e in range(E):
        outs[e] = (
            res.results[e]["outt"].transpose(2, 0, 1).reshape(npad, D)
        )
    final = outs[idx, inv[idx, np.arange(T)[:, None]]]
    return final, res


def kernel(**inputs) -> np.ndarray:
    out, _ = _run(inputs, trace=False)
    return out



# revision 3
# speedup vs baseline: 1.3839x; 1.3839x over previous
"""Expert-parallel MoE ConditionalFeedForward (SwiGLU) for 8 Trainium2 cores.

Math (per token t, selected expert e):
    out[t] = (silu(x[t] @ w1[e].T) * (x[t] @ w3[e].T)) @ w2[e]

Strategy: one expert per NeuronCore (8 experts / 8 cores). The host routes
tokens to experts (gather), each core runs the dense SwiGLU FFN for its
expert's tokens, and the host scatters results back into [T, top_k, D].

All matmuls run as fp8e4 (e4m3) DoubleRow pairs (K=256 per instruction at
0.5 cycles/row — 4x the fp32r MAC rate). Accuracy is recovered with a
3-term residual expansion per GEMM: every operand A is split host- or
chip-side into A_hi = fp8(A) and A_lo = fp8(A - A_hi), and the product is
A_hi.B_hi + A_lo.B_hi + A_hi.B_lo (the eps^2 cross term is dropped), which
lands ~2e-3 relative error at 0.75x the fp32r cycle count.

Scaling: fp8e4 here is the inf-variant e4m3 (max finite 240). The hidden
activation g = silu(x1)*x3 (|g| up to ~2e4) is kept as g' = g*2^-7 on chip,
w2 is pre-scaled by 2^5 on host, and the final PSUM->SBUF copy multiplies
by 4 to restore out = g @ w2.
"""

import numpy as np
import ml_dtypes

import concourse.bacc as bacc
import concourse.mybir as mybir
from concourse.bass_utils import run_bass_kernel_spmd
from concourse.tile import TileContext

# Problem constants (nn_ConditionalFeedForward: dim=1024, hidden=2816, 8 experts, top-2)
T = 2048
D = 1024
H = 2816
E = 8
TOPK = 2
ND = D // 128    # 8 d-tiles
NH = H // 128    # 22 h-tiles
NJ1 = ND // 2    # 4 DoubleRow K-pairs, stage 1
NJ2 = NH // 2    # 11 DoubleRow K-pairs, stage 2

F32 = mybir.dt.float32
F8 = mybir.dt.float8e4
E4 = ml_dtypes.float8_e4m3
DRM = mybir.MatmulPerfMode.DoubleRow
GS = 2.0 ** -7    # on-chip g scale (keeps |g'| < 240)
WS = 2.0 ** 5     # host-side w2 scale
OS = 1.0 / (GS * WS)  # output restore scale (= 4)

_BUILD_CACHE: dict[tuple, object] = {}


def _build(npad: int, loop_n: int = 0):
    """Bass program for one core: fp8 DoubleRow SwiGLU FFN over npad tokens.

    loop_n > 0 wraps the body in a hardware loop (benchmarking only).
    """
    key = (npad, loop_n)
    if key in _BUILD_CACHE:
        return _BUILD_CACHE[key]
    # token chunks <= 512 (one PSUM bank each)
    nchunks = -(-npad // 512)
    base = npad // nchunks
    sizes = [base + (1 if i < npad % nchunks else 0) for i in range(nchunks)]
    chunks, off = [], 0
    for sz in sizes:
        chunks.append((off, sz))
        off += sz

    nc = bacc.Bacc("TRN2", target_bir_lowering=False)
    xt = nc.dram_tensor("xt", [128, 2, ND, npad], F8, kind="ExternalInput")
    w13 = nc.dram_tensor("w13", [NH, 128, 2, 2, ND, 128], F8, kind="ExternalInput")
    w2t = nc.dram_tensor("w2t", [ND, 128, 2, NH, 128], F8, kind="ExternalInput")
    outt = nc.dram_tensor("outt", [ND, 128, npad], F32, kind="ExternalOutput")

    import contextlib

    ALU = mybir.AluOpType
    TERMS1 = ((0, 0), (1, 0), (0, 1))  # (w term, x term): hi.hi, lo.hi, hi.lo

    with TileContext(nc) as tc:
        with (
            tc.For_i(0, loop_n, 1) if loop_n else contextlib.nullcontext(),
            tc.tile_pool(name="xg", bufs=1) as xg_pool,
            tc.tile_pool(name="w13p", bufs=4) as w13_pool,
            tc.tile_pool(name="w2p", bufs=3) as w2_pool,
            tc.tile_pool(name="tmp", bufs=4) as tmp_pool,
        ):
            x_sb = xg_pool.tile([128, 2, ND, npad], F8)
            # x_hi per d-pair first (gates the first matmuls), then x_lo
            for j in range(NJ1):
                nc.sync.dma_start(
                    x_sb[:, 0, 2 * j:2 * j + 2, :], xt[:, 0, 2 * j:2 * j + 2, :]
                )
            nc.sync.dma_start(x_sb[:, 1], xt[:, 1])
            gh_sb = xg_pool.tile([128, NH, npad], F8, tag="gh")
            gl_sb = xg_pool.tile([128, NH, npad], F8, tag="gl")

            # stage-2 weight prefetch (filled during stage 1)
            w2_tiles = {}

            def load_w2(dt):
                t = w2_pool.tile([128, 2, NH, 128], F8, name=f"w2_{dt}", tag="w2")
                nc.sync.dma_start(t[:], w2t[dt])
                w2_tiles[dt] = t

            # ---- stage 1: g'[h, n] = silu(w1.T x)[h, n] * (w3.T x)[h, n] * GS
            with tc.tile_pool(name="ps1", bufs=3, space="PSUM") as ps1_pool, \
                 tc.tile_pool(name="ps2", bufs=2, space="PSUM") as ps2_pool:
                for h in range(NH):
                    if h in (8, 12, 16):
                        load_w2({8: 0, 12: 1, 16: 2}[h])
                    wt = w13_pool.tile([128, 2, 2, ND, 128], F8, tag="wt")
                    if h == 0:
                        # fine-grained first load so matmul 0 starts early
                        for s in range(2):
                            for t in range(2):
                                nc.sync.dma_start(wt[:, s, t], w13[h, :, s, t])
                    else:
                        nc.sync.dma_start(wt[:, 0], w13[h, :, 0])
                        nc.sync.dma_start(wt[:, 1], w13[h, :, 1])
                    ps = {
                        (s, ci): ps1_pool.tile([128, cl], F32, tag=f"ps{s}{ci}",
                                               name=f"ps_{s}_{ci}")
                        for s in range(2) for ci, (cs, cl) in enumerate(chunks)
                    }
                    for s in range(2):
                        for ci, (cs, cl) in enumerate(chunks):
                            k = 0
                            for tw, rx in TERMS1:
                                for j in range(NJ1):
                                    nc.tensor.matmul(
                                        ps[s, ci][:],
                                        wt[:, s, tw, 2 * j:2 * j + 2, :],
                                        x_sb[:, rx, 2 * j:2 * j + 2, cs:cs + cl],
                                        start=(k == 0),
                                        stop=(k == 3 * NJ1 - 1),
                                        perf_mode=DRM,
                                    )
                                    k += 1
                    for ci, (cs, cl) in enumerate(chunks):
                        t_silu = tmp_pool.tile([128, cl], F32, tag=f"silu{ci}")
                        nc.scalar.activation(
                            t_silu[:], ps[0, ci][:], mybir.ActivationFunctionType.Silu
                        )
                        gtmp = tmp_pool.tile([128, cl], F32, tag=f"gt{ci}")
                        nc.vector.scalar_tensor_tensor(
                            gtmp[:], t_silu[:], GS, ps[1, ci][:],
                            op0=ALU.mult, op1=ALU.mult,
                        )
                        nc.scalar.copy(gh_sb[:, h, cs:cs + cl], gtmp[:])
                        nc.vector.scalar_tensor_tensor(
                            gl_sb[:, h, cs:cs + cl], gtmp[:], 1.0,
                            gh_sb[:, h, cs:cs + cl],
                            op0=ALU.mult, op1=ALU.subtract,
                        )

                # ---- stage 2: out[dt, n] = 4 * sum_h w2'[h, dt].T g'[h, n] ----
                # last K-pair (h=20,21) goes last so dt=0 can start while the
                # tail of stage 1 still quantizes g
                order = [(tm, j) for tm in range(3) for j in range(NJ2 - 1)]
                order += [(tm, NJ2 - 1) for tm in range(3)]
                MV = (None, None, None)
                for dt in range(ND):
                    if dt + 3 < ND:
                        load_w2(dt + 3)
                    w2_sb = w2_tiles.pop(dt)
                    MV = (gh_sb, gh_sb, gl_sb)
                    TW = (0, 1, 0)
                    for ci, (cs, cl) in enumerate(chunks):
                        ps_o = ps2_pool.tile([128, cl], F32, tag=f"o{ci}", name="o_ps")
                        for k, (tm, j) in enumerate(order):
                            nc.tensor.matmul(
                                ps_o[:],
                                w2_sb[:, TW[tm], 2 * j:2 * j + 2, :],
                                MV[tm][:, 2 * j:2 * j + 2, cs:cs + cl],
                                start=(k == 0),
                                stop=(k == len(order) - 1),
                                perf_mode=DRM,
                            )
                        # split the drain: copy+DMA halves overlap the next MMs
                        half = cl // 2
                        for oi, (ho, hl) in enumerate([(0, half), (half, cl - half)]):
                            o_sb = tmp_pool.tile([128, hl], F32, tag=f"ot{ci}{oi}",
                                                 name="o_sb")
                            nc.scalar.activation(
                                o_sb[:], ps_o[:, ho:ho + hl],
                                mybir.ActivationFunctionType.Copy, scale=OS,
                            )
                            nc.sync.dma_start(
                                outt[dt, :, cs + ho:cs + ho + hl], o_sb[:])
    nc.compile()
    _BUILD_CACHE[key] = nc
    return nc


def _route(expert_indices: np.ndarray):
    """Per-expert token lists, padded count, and an inverse position map."""
    toks = []
    for e in range(E):
        mask = (expert_indices == e).any(axis=1)
        toks.append(np.flatnonzero(mask))
    maxc = max(len(tk) for tk in toks)
    npad = max(8, -(-maxc // 8) * 8)
    inv = np.zeros((E, T), dtype=np.int64)
    for e, tk in enumerate(toks):
        inv[e, tk] = np.arange(len(tk))
    return toks, npad, inv


def _q8(a):
    """e4m3 (inf variant, max 240) quantize via ml_dtypes, saturating."""
    return np.clip(a, -240.0, 240.0).astype(E4)


def _core_in_map(e, x, w1, w2, w3, tk, npad):
    """Host-side fp8 hi/lo packing for one expert's core."""
    xg = np.zeros((npad, D), dtype=np.float32)
    xg[: len(tk)] = x[tk]
    xh = _q8(xg)
    xl = _q8(xg - xh.astype(np.float32))
    # xt[i, r, d, n] = x_r[n, d*128 + i]
    xr = np.stack([xh, xl])  # [2, npad, D]
    xt = np.ascontiguousarray(
        xr.reshape(2, npad, ND, 128).transpose(3, 0, 2, 1)
    )
    # w13[h, i, s, t, d, j] = q_t(w_s)[h*128 + j, d*128 + i]
    w1h = _q8(w1[e]); w1l = _q8(w1[e] - w1h.astype(np.float32))
    w3h = _q8(w3[e]); w3l = _q8(w3[e] - w3h.astype(np.float32))
    wst = np.stack([np.stack([w1h, w1l]), np.stack([w3h, w3l])])  # [s, t, H, D]
    w13 = np.ascontiguousarray(
        wst.reshape(2, 2, NH, 128, ND, 128).transpose(2, 5, 0, 1, 4, 3)
    )
    # w2t[dt, i, t, h, j] = q_t(w2*WS)[h*128 + i, dt*128 + j]
    w2s = w2[e] * WS
    w2h = _q8(w2s); w2l = _q8(w2s - w2h.astype(np.float32))
    w2p = np.stack([w2h, w2l])  # [t, H, D]
    w2e = np.ascontiguousarray(
        w2p.reshape(2, NH, 128, ND, 128).transpose(3, 2, 0, 1, 4)
    )
    return {"xt": xt, "w13": w13, "w2t": w2e}


def _prep_in_maps(inputs):
    x = np.ascontiguousarray(inputs["x"], dtype=np.float32)
    idx = np.asarray(inputs["expert_indices"])
    w1 = np.asarray(inputs["w1"], dtype=np.float32)
    w2 = np.asarray(inputs["w2"], dtype=np.float32)
    w3 = np.asarray(inputs["w3"], dtype=np.float32)
    toks, npad, inv = _route(idx)
    in_maps = [
        _core_in_map(e, x, w1, w2, w3, toks[e], npad) for e in range(E)
    ]
    return in_maps, toks, npad, inv


def _run(inputs, trace=False):
    idx = np.asarray(inputs["expert_indices"])
    in_maps, toks, npad, inv = _prep_in_maps(inputs)
    nc = _build(npad)

    res = run_bass_kernel_spmd(
        nc, in_maps, core_ids=list(range(E)), trace=trace,
        **({"stitch_traces": True} if trace else {}),
    )

    # outs[e, n, dd] = outt[dt, i, n] with dd = dt*128 + i
    outs = np.empty((E, npad, D), dtype=np.float32)
    for e in range(E):
        outs[e] = (
            res.results[e]["outt"].transpose(2, 0, 1).reshape(npad, D)
        )
    final = outs[idx, inv[idx, np.arange(T)[:, None]]]
    return final, res


def kernel(**inputs) -> np.ndarray:
    out, _ = _run(inputs, trace=False)
    return out
